# revision 12
# baseline (speedup 1.0000x reference)
"""Trainium2 Bass kernel for per-pixel dynamic 3D filtering.

    out[b, h, w, o] = sum_k patches[b, h, w, k] * f[b, h, w, k, o]

with patches = im2col(x) over a 3x3 spatial window (zero-padded SAME) and
3 time steps, k ordered (kh, kw, t), K=27, C_out=16, B=8, H=W=192.

Sharding: pure data parallel over batch — core c computes image c.

v3 design (this file). The problem is HBM-bound on streaming f (64MB/core in
f32), so f and patches are staged host-side in float16 — harness gate is
rel_err < 2e-2, fp16 staging costs ~7e-3 worst-case — halving the dominant
DMA stream. To keep the DVE from becoming the new bottleneck, the fused
multiply+prefix-scan custom op gets a hand-written 2X_1PORT uop program
(2 elems/cycle on packed fp16 pairs), selected via byte-36 perf_max:

  blk0: m0 = f_lo*p_lo          blk1: m1 = f_hi*p_hi (pair via delay chains)
  blk2: pair = m0+m1            blk3: s += pair  (1-cycle recurrence)
  blk4: out_lo = s - m1 (delayed m1) ; s rides delay chain 2 to the
  write mux -> WR0_LO/WR0_HI packed fp16 pair per cycle.

Per-core layout (one image): pixels map to SBUF partitions in 8h x 16w
blocks (partition p = dh*16 + dw holds 12 consecutive pixels), and the
per-partition f slab is host-permuted to (o, g, k) so the whole supertile
scan stream for all 16 output channels is ONE contiguous step-1 affine dim
(5184 fp16 = 2592 packed pairs/partition). Prefix stored at +2 elements
(4B-aligned for 2x) behind a zeroed 2-elem pad; every (pixel, o) tap-sum is
prefix[end] - prefix[end-27] — uniform across pixel AND o-row boundaries —
recovered by ONE strided tensor_sub into a (o, g)-ordered fp16 out tile.

DMA per supertile: f 1.33MB + patches 83KB + out 49KB ~= 1.46MB -> ~4.1us
at the ~360GB/s HBM/NC roofline; DVE 2x scan ~2.9us + sub 0.25us hides
underneath. 24 supertiles -> ~100us/core target (vs 207us f32 baseline).
"""

import os
from contextlib import ExitStack

import numpy as np

# ---- problem constants (hardcoded per contract) ---------------------------
B, T, H, W = 8, 3, 192, 192
K = 3
PAD = K // 2
KK = T * K * K  # 27
CO = 16
N_CORES = 8

# supertile geometry
DH, DW, G = 8, 16, 12  # partitions = DH*DW = 128; per-partition pixels = G
P = DH * DW  # 128
N_ST = H // DH  # 24 supertiles per image
FFREE = CO * G * KK  # 5184 fp16 per partition per supertile, (o, g, k) order
PFREE = G * KK  # 324 patch fp16 per partition per supertile, (g, k) order
OFREE = CO * G  # 192 out fp16 per partition per supertile, (o, g) order
PADE = 2  # zeroed pad elems ahead of the prefix (keeps 2x out 4B-aligned)


def _im2col_batch(x: np.ndarray) -> np.ndarray:
    """x: (B, T, H, W) f32 -> patches (B, H, W, 27), k ordered (kh, kw, t)."""
    Bb, Tt, Hh, Ww = x.shape
    xp = np.pad(x, ((0, 0), (0, 0), (PAD, PAD), (PAD, PAD)))
    cols = [
        xp[:, t, i : i + Hh, j : j + Ww]
        for i in range(K)
        for j in range(K)
        for t in range(Tt)
    ]
    return np.stack(cols, axis=-1).astype(np.float32)


XFREE = T * K * 16  # 144: per-partition per-supertile x-window (wl padded 14->16)


def _xpp_batch(x: np.ndarray) -> np.ndarray:
    """Per-partition x windows: (B,T,H,W) -> (B, N_ST*P, 144) fp16, layout
    (t, kh, wl) per partition; value = xp[t, 8s+dh+kh, dw*12+wl], wl<14."""
    xp = np.pad(x, ((0, 0), (0, 0), (PAD, PAD), (PAD, PAD))).astype(np.float32)
    out = np.zeros((x.shape[0], N_ST, DH, DW, T, K, 16), np.float32)
    rows = np.arange(H).reshape(N_ST, DH)
    cols = (np.arange(DW) * G)[:, None] + np.arange(14)[None, :]
    for kh in range(K):
        sub = xp[:, :, rows + kh, :][:, :, :, :, cols]  # (B,T,NST,DH,DW,14)
        out[..., kh, :14] = np.moveaxis(sub, 1, 4)
    return out.reshape(x.shape[0], N_ST * P, XFREE).astype(np.float16)


def _register_custom_op():
    """Register DYNF_MACSCAN2X_ANT: out = running_sum(in0 * in1) along the
    free stream (inclusive prefix scan of the product). The REGULAR (1x)
    program comes from lower(); the 2X_1PORT program is hand-written (the
    Spec compiler does not emit perf-mode variants) and seeded into the
    compile cache so table-gen picks it up."""
    import concourse.dve_ops as dve_ops
    from concourse.dve_spec import AluOp, Spec, Src0, Src1, lower, scan
    from concourse.dve_uop import (
        AluInp,
        DelayInp,
        DveOpSpec,
        InpSel,
        OutPath,
        OutSel,
        Trigger,
        UopConfig,
    )

    name = "DYNF_MACSCAN2X_ANT"
    for op in dve_ops.OPS:
        if op.name == name:
            return op

    def _ref(in0, in1, c0, c1, c2):
        a = np.asarray(in0, np.float32)
        b = np.asarray(in1, np.float32)
        prod = a.reshape(a.shape[0], -1) * b.reshape(b.shape[0], -1)
        return np.cumsum(prod, axis=1, dtype=np.float32).reshape(a.shape)

    spec = Spec(body=scan(AluOp.ADD, Src0 * Src1), reference=_ref)
    uops_1x = lower(spec, ver="v3")

    def _mk_body():
        u = UopConfig()
        for lane, src in (
            (1, InpSel.SRC_0),
            (2, InpSel.SRC_1),
            (3, InpSel.SRC_0_HI),
            (4, InpSel.SRC_1_HI),
        ):
            u.enable_input(src, lane)
        dp = u.datapath_config
        dp[0].enable_alu(AluOp.MULTIPLY, AluInp.PREV_DELAY_0, AluInp.PREV_DELAY_1)
        dp[0].pass_through_delay(2, 3)  # carry the hi pair to blk1
        dp[1].enable_alu(AluOp.MULTIPLY, AluInp.PREV_DELAY_2, AluInp.PREV_DELAY_3)
        dp[1].enable_delay_from_src(DelayInp.PREV_ALU_OUT, 0)  # m0
        dp[2].enable_alu(AluOp.ADD, AluInp.PREV_ALU_OUT, AluInp.PREV_DELAY_0)
        dp[2].enable_delay_from_src(DelayInp.PREV_ALU_OUT, 1)  # m1
        dp[3].enable_alu(AluOp.ADD, AluInp.CURR_ALU_OUT, AluInp.PREV_ALU_OUT)
        dp[3].pass_through_delay(1)
        dp[4].enable_alu(AluOp.SUBTRACT, AluInp.PREV_ALU_OUT, AluInp.PREV_DELAY_1)
        dp[4].enable_delay_from_src(DelayInp.PREV_ALU_OUT, 2)  # s (hi result)
        for bkt in (5, 6, 7):
            dp[bkt].pass_through_alu()
            dp[bkt].pass_through_delay(2)
        return u

    steady = _mk_body()
    steady.enable_output(OutSel.ALU_OUT, OutPath.WR0_LO)
    steady.enable_output(OutSel.DELAY_2, OutPath.WR0_HI)
    steady.require_inp0 = 1
    steady.require_inp1 = 1
    steady.trigger = (Trigger.SRC_TENSOR_DONE, Trigger.NONE, Trigger.NONE)
    steady.next_uop = (0, 0, 0)

    # seed: zero blk3's accumulator flop (BITWISE_XOR(x, x) == +0.0 for any
    # bit pattern, incl. NaN garbage) in one non-consuming cycle, then run.
    seed = _mk_body()
    seed.datapath_config[3].enable_alu(
        AluOp.BITWISE_XOR, AluInp.CURR_ALU_OUT, AluInp.CURR_ALU_OUT
    )
    seed.require_inp0 = 0
    seed.require_inp1 = 0
    seed.repeat_count = 1
    seed.trigger = (Trigger.COUNT, Trigger.NONE, Trigger.NONE)
    seed.next_uop = (1, 0, 0)

    row = dve_ops._CUSTOM_DVE_ROW_BASE + len(dve_ops.OPS)
    assert row < 0x20
    global _HAS_2X
    try:
        spec2 = DveOpSpec(
            name=name,
            opcode=row,
            uops=uops_1x,
            uops_2x=[seed, steady],
            perf_max=1,
            rd1_en=True,
        )
        spec2.validate("v3")
        _HAS_2X = True
    except Exception:
        # 2x program rejected (concourse drift?) -> register 1x-only; the
        # builder must then NOT set byte-36 perf bits (no 2x table slot).
        spec2 = DveOpSpec(name=name, opcode=row, uops=uops_1x, rd1_en=True)
        _HAS_2X = False
    shas = {"v3": spec2.sha("v3")}
    op = dve_ops.DveOp(name, spec, subdim=False, uops_sha=shas)
    dve_ops.OPS.append(op)
    dve_ops._SUB_OPCODE_FOR_NAME[name] = row
    dve_ops.CUSTOM_DVE_SPECS[name] = spec
    dve_ops._COMPILE_CACHE[(name, "v3")] = spec2
    return op


def _build_program_v3(reps: int = 1, mode: str = "full"):
    """fp16 wide-scan kernel; mode: "full" | "dma" (no compute, ships f's
    first OFREE elems as out) — diagnostics."""
    import concourse.bacc as bacc
    import concourse.tile as tile
    from concourse import mybir

    f16 = mybir.dt.float16
    mac_op = _register_custom_op()
    perf = os.environ.get("DYNF3_PERF", "2x") == "2x" and _HAS_2X
    patch_mode = os.environ.get("DYNF3_PATCH", "packed")

    nc = bacc.Bacc("TRN2", debug=False, enable_asserts=False)

    f_ap = nc.dram_tensor("f_in", (N_ST * P, FFREE), f16, kind="ExternalInput").ap()
    pfree_in = XFREE if patch_mode == "expand" else PFREE
    p_ap = nc.dram_tensor(
        "p_in", (N_ST * P, pfree_in), f16, kind="ExternalInput"
    ).ap()
    o_ap = nc.dram_tensor("o_out", (N_ST * P, OFREE), f16, kind="ExternalOutput").ap()

    fbufs = int(os.environ.get("DYNF3_FBUFS", "3"))
    prefbufs = int(os.environ.get("DYNF3_PREFBUFS", "3"))
    obufs = int(os.environ.get("DYNF3_OBUFS", "6"))
    nsplit = int(os.environ.get("DYNF3_SPLIT", "2"))
    alloc_mode = os.environ.get("DYNF3_POOL_ALLOC", "stack")

    with tile.TileContext(nc, pool_alloc_mode=alloc_mode) as tc, ExitStack() as ctx:
        fpool = ctx.enter_context(tc.tile_pool(name="fpool", bufs=fbufs))
        ppool = ctx.enter_context(tc.tile_pool(name="ppool", bufs=3))
        prefpool = ctx.enter_context(tc.tile_pool(name="prefpool", bufs=prefbufs))
        opool = ctx.enter_context(tc.tile_pool(name="opool", bufs=obufs))

        zpool = ctx.enter_context(tc.tile_pool(name="zpool", bufs=1))
        zerot = zpool.tile([P, PADE], f16)
        nc.vector.memset(zerot[:], 0.0)

        if mode == "dve":
            # pure DVE throughput probe: resident f/p tiles, scans + subs only
            ft0 = fpool.tile([P, FFREE], f16)
            nc.sync.dma_start(ft0[:], f_ap[0:P, :])
            pt0 = ppool.tile([P, PFREE], f16, tag="pt")
            nc.sync.dma_start(pt0[:], p_ap[0:P, :PFREE])
            APc = type(ft0[:])
            for _ in range(reps):
                for s in range(N_ST):
                    rows = slice(s * P, (s + 1) * P)
                    pref = prefpool.tile([P, FFREE + PADE], f16)
                    nc.scalar.copy(pref[:, 0:PADE], zerot[:])
                    fa, pa, pra = ft0[:], pt0[:], pref[:]
                    in0 = APc(
                        fa.tensor, fa.offset, [list(fa.ap[0]), [PFREE, CO], [1, PFREE]]
                    )
                    in1 = APc(
                        pa.tensor, pa.offset, [list(pa.ap[0]), [0, CO], [1, PFREE]]
                    )
                    outp = APc(
                        pra.tensor,
                        pra.offset + PADE,
                        [list(pra.ap[0]), [PFREE, CO], [1, PFREE]],
                    )
                    inst = nc.vector._custom_dve(mac_op, out=outp, in0=in0, in1=in1)
                    if perf:
                        inst.ins.perf_max = 1
                    ot = opool.tile([P, OFREE], f16)
                    e1 = APc(
                        pra.tensor,
                        pra.offset + PADE + KK - 1,
                        [list(pra.ap[0]), [KK, OFREE]],
                    )
                    e0 = APc(
                        pra.tensor,
                        pra.offset + PADE - 1,
                        [list(pra.ap[0]), [KK, OFREE]],
                    )
                    nc.vector.tensor_sub(ot[:], e1, e0)
                    nc.scalar.dma_start(o_ap[rows, :], ot[:])
            reps = 0  # skip the streaming loop; compile after context exit

        for _ in range(reps):
            for s in range(N_ST):
                rows = slice(s * P, (s + 1) * P)
                ft = fpool.tile([P, FFREE], f16)
                hw_elems = FFREE // nsplit
                for h in range(nsplit):
                    nc.sync.dma_start(
                        ft[:, h * hw_elems : (h + 1) * hw_elems],
                        f_ap[rows, h * hw_elems : (h + 1) * hw_elems],
                    )
                if patch_mode == "expand":
                    xt = ppool.tile([P, XFREE], f16, tag="xt")
                    nc.sync.dma_start(xt[:], p_ap[rows, :])
                    # expand windows -> (g, kh, kw, t)-packed patches on ACT
                    # (idle engine): pt[g, kh, kw, t] = xt[t, kh, g+kw]
                    pt = ppool.tile([P, PFREE], f16, tag="pt")
                    pt5 = pt[:].rearrange(
                        "p (g kh kw t) -> p kh g kw t", g=G, kh=K, kw=K, t=T
                    )
                    xta = xt[:]
                    APx = type(xta)
                    for kh in range(K):
                        src = APx(
                            xta.tensor,
                            xta.offset + kh * 16,
                            [list(xta.ap[0]), [1, G], [1, K], [K * 16, T]],
                        )
                        nc.scalar.copy(pt5[:, kh], src)
                else:
                    pt = ppool.tile([P, PFREE], f16, tag="pt")
                    nc.sync.dma_start(pt[:], p_ap[rows, :])

                if mode == "dma":
                    nc.scalar.dma_start(o_ap[rows, :], ft[:, :OFREE])
                    continue

                # prefix tile: [pad(2) | scan of 5184 products], all fp16
                pref = prefpool.tile([P, FFREE + PADE], f16)
                nc.scalar.copy(pref[:, 0:PADE], zerot[:])

                APc = type(ft[:])
                fa, pa, pra = ft[:], pt[:], pref[:]
                in0 = APc(
                    fa.tensor, fa.offset, [list(fa.ap[0]), [PFREE, CO], [1, PFREE]]
                )
                in1 = APc(
                    pa.tensor, pa.offset, [list(pa.ap[0]), [0, CO], [1, PFREE]]
                )
                outp = APc(
                    pra.tensor,
                    pra.offset + PADE,
                    [list(pra.ap[0]), [PFREE, CO], [1, PFREE]],
                )
                inst = nc.vector._custom_dve(mac_op, out=outp, in0=in0, in1=in1)
                if perf:
                    # byte-36[7:6] on the inner ISA instruction: engine picks
                    # the 2X_1PORT uop program when the APs qualify.
                    inst.ins.perf_max = 1

                # segment sums: prefix[end] - prefix[end-27], ends every 27
                # elements throughout the whole (o, g) stream; out in stream
                # (o, g) order.
                ot = opool.tile([P, OFREE], f16)
                oa = ot[:]
                e1 = APc(
                    pra.tensor,
                    pra.offset + PADE + KK - 1,
                    [list(pra.ap[0]), [KK, OFREE]],
                )
                e0 = APc(
                    pra.tensor, pra.offset + PADE - 1, [list(pra.ap[0]), [KK, OFREE]]
                )
                nc.vector.tensor_sub(oa, e1, e0)

                # out-DMA on the ACT HWDGE ring: keeps the sync-engine ring a
                # pure f/p prefetch stream.
                nc.scalar.dma_start(o_ap[rows, :], ot[:])

    nc.compile()
    return nc


_NC_CACHE = None

# test harness introspection: last BassKernelResults (exec_time_ns when traced)
LAST_RESULTS = None

# set by _register_custom_op: whether the 2x table row registered OK
_HAS_2X = False


def build_program(reps: int = 1, mode: str = "full"):
    return _build_program_v3(reps, mode=mode)


def _get_nc():
    global _NC_CACHE
    if _NC_CACHE is None:
        _NC_CACHE = build_program(1)
    return _NC_CACHE


def prepare_in_maps(x: np.ndarray, f: np.ndarray) -> list[dict]:
    """Host-side staging: per-core {f_in, p_in} in the device DRAM layouts
    (fp16; f permuted per-partition to (o, g, k), patches packed (g, k))."""
    x = np.asarray(x, dtype=np.float32)
    f = np.asarray(f, dtype=np.float32)
    assert x.shape == (B, T, H, W) and f.shape == (B, H, W, KK, CO)

    if os.environ.get("DYNF3_PATCH", "packed") == "expand":
        p_blk = _xpp_batch(x)  # (B, N_ST*P, 144) fp16
    else:
        patches = _im2col_batch(x)  # (B, H, W, 27)
        p_blk = (
            patches.reshape(B, N_ST, DH, DW, G, KK)
            .reshape(B, N_ST * P, PFREE)
            .astype(np.float16)
        )
    f_blk = (
        f.reshape(B, N_ST, DH, DW, G, KK, CO)
        .transpose(0, 1, 2, 3, 6, 4, 5)  # (b, s, dh, dw, o, g, k)
        .astype(np.float16)
        .reshape(B, N_ST * P, FFREE)
    )
    return [{"f_in": f_blk[c], "p_in": p_blk[c]} for c in range(N_CORES)]


def kernel(x: np.ndarray, f: np.ndarray) -> np.ndarray:
    import concourse.bass_utils as bass_utils

    nc = _get_nc()
    in_maps = prepare_in_maps(x, f)
    res = bass_utils.run_bass_kernel_spmd(nc, in_maps, core_ids=list(range(N_CORES)))
    global LAST_RESULTS
    LAST_RESULTS = res

    out = np.empty((B, H, W, CO), dtype=np.float32)
    for c in range(N_CORES):
        o = np.asarray(res.results[c]["o_out"])  # (N_ST*P, OFREE) fp16
        o = o.astype(np.float32).reshape(N_ST, DH, DW, CO, G)
        out[c] = o.transpose(0, 1, 2, 4, 3).reshape(H, W, CO)
    return out
